# revision 3
# baseline (speedup 1.0000x reference)
"""Trainium2 Bass kernel for a 2-layer Chebyshev KAN.

Computation (degree-5 Chebyshev KAN, matching the reference):
    t1  = tanh(x)
    y1  = sum_d T_d(tanh(t1)) @ C1_d.T + t1 @ Wb1.T + b1
    h   = SiLU(LayerNorm(y1))
    out = sum_d T_d(tanh(h)) @ C2_d.T + h @ Wb2.T + b2

Strategy: data-parallel over the batch dim across 8 NeuronCores (2048 rows
per core); weights replicated, pre-transposed + cast to bf16 on the host and
kept resident in SBUF. The T_0 == 1 term is folded into an effective bias on
the host, leaving 6 [in,out] matmul matrices per layer (base + d=1..5).

On-chip dataflow per 256-row chunk (2 partition tiles of 128 rows):
  x tile -> tanh (ACT) -> PE-transpose each 128x128 block to feature-major ->
  tanh again (ACT, from PSUM) -> Chebyshev recurrence in bf16 (DVE) ->
  6 stationary [128,128] bf16 tiles per i-block -> PE matmuls accumulate
  y1[128b, 1024] in PSUM -> bias add + LayerNorm (bn_stats) + SiLU in
  batch-major layout -> PE-transpose h -> layer 2 identically -> DMA out.
"""

import math

import numpy as np
import ml_dtypes

import concourse.bass as bass
import concourse.tile as tile
from concourse import bacc, mybir
from concourse.bass_utils import run_bass_kernel_spmd
from concourse.masks import make_identity

N_CORES = 8
B, D0, D1, D2 = 16384, 1024, 1024, 512
BC = B // N_CORES            # rows per core
NBT = BC // 128              # 16 partition tiles per core
CHUNK_BT = 2                 # partition tiles processed per chunk
NCHUNK = NBT // CHUNK_BT
FD = CHUNK_BT * 128          # free-dim width of feature-major tiles
LN_EPS = 1e-5

F32 = mybir.dt.float32
BF16 = mybir.dt.bfloat16
AF = mybir.ActivationFunctionType
ALU = mybir.AluOpType


def _bcast_row(nc, pool, vec_ap, n, name):
    """Load a [n] DRAM vector broadcast across all 128 partitions."""
    t = pool.tile([128, n], F32, name=name)
    src = bass.AP(tensor=vec_ap.tensor, offset=vec_ap.offset,
                  ap=[[0, 128], list(vec_ap.ap[0])])
    nc.gpsimd.dma_start(out=t[:], in_=src)
    return t


def _cheb_fill(nc, cheb, xn, tmpp, upool):
    """Fill cheb[:, 1..5] (bf16) with T_1..T_5 of xn (f32, shape [128, FD])."""
    T = [None] + [cheb[:, k] for k in range(1, 6)]
    # T1 = xn
    nc.vector.tensor_copy(T[1], xn)
    # T2 = 2*xn^2 - 1   (Square(sqrt(2)*xn) = 2 xn^2 on ACT, then -1 on DVE)
    sq = tmpp.tile([128, CHUNK_BT, 128], F32, tag="sq", name="sq")
    nc.scalar.activation(sq[:], xn, AF.Square, scale=math.sqrt(2.0))
    nc.vector.tensor_scalar(T[2], sq[:], 1.0, None, op0=ALU.subtract)
    # T_k = (2*T1)*T_{k-1} - T_{k-2}
    for k in (3, 4, 5):
        u = upool.tile([128, CHUNK_BT, 128], BF16, tag="u", name=f"u{k}")
        nc.vector.scalar_tensor_tensor(u[:], T[1], 2.0, T[k - 1],
                                       op0=ALU.mult, op1=ALU.mult)
        nc.vector.tensor_tensor(T[k], u[:], T[k - 2], op=ALU.subtract)


def _kernel_body(tc, out_d, x_d, w1_d, w2_d, b1_d, b2_d, g_d, be_d):
    nc = tc.nc
    import contextlib
    ctx = contextlib.ExitStack()
    with ctx:
        consts = ctx.enter_context(tc.tile_pool(name="consts", bufs=1))
        wpool = ctx.enter_context(tc.tile_pool(name="wpool", bufs=1))
        xpool = ctx.enter_context(tc.tile_pool(name="xpool", bufs=3))
        tmpp = ctx.enter_context(tc.tile_pool(name="tmpp", bufs=3))
        upool = ctx.enter_context(tc.tile_pool(name="upool", bufs=4))
        chebp = ctx.enter_context(tc.tile_pool(name="chebp", bufs=3))
        ypool = ctx.enter_context(tc.tile_pool(name="ypool", bufs=3))
        statp = ctx.enter_context(tc.tile_pool(name="statp", bufs=6))
        opool = ctx.enter_context(tc.tile_pool(name="opool", bufs=3))
        ps_y1 = ctx.enter_context(tc.tile_pool(name="ps_y1", bufs=2, space="PSUM"))
        ps_tr = ctx.enter_context(tc.tile_pool(name="ps_tr", bufs=2, space="PSUM"))
        ps_y2 = ctx.enter_context(tc.tile_pool(name="ps_y2", bufs=2, space="PSUM"))

        ident = consts.tile([128, 128], F32, name="ident")
        make_identity(nc, ident[:])
        eps_t = consts.tile([128, 1], F32, name="eps_t")
        nc.vector.memset(eps_t[:], LN_EPS)

        b1_t = _bcast_row(nc, consts, b1_d, D1, "b1_t")
        b2_t = _bcast_row(nc, consts, b2_d, D2, "b2_t")
        g_t = _bcast_row(nc, consts, g_d, D1, "g_t") if g_d is not None else None
        be_t = _bcast_row(nc, consts, be_d, D1, "be_t") if be_d is not None else None

        # Resident weights, contraction dim (input features) on partitions.
        w1_sb = wpool.tile([128, 6, 8, D1], BF16, name="w1_sb")
        w2_sb = wpool.tile([128, 6, 8, D2], BF16, name="w2_sb")
        for i in range(8):
            for d in range(6):
                nc.sync.dma_start(out=w1_sb[:, d, i, :],
                                  in_=w1_d[d, i * 128:(i + 1) * 128, :])
                nc.sync.dma_start(out=w2_sb[:, d, i, :],
                                  in_=w2_d[d, i * 128:(i + 1) * 128, :])

        def cheb_layer(src_tiles, w_sb, ps_pool, dout, ps_tag):
            """src_tiles: CHUNK_BT tiles [128, 1024] f32 (layer input, batch-major).
            Returns list of CHUNK_BT PSUM tiles [128, dout] accumulated."""
            nbank = dout // 512
            ps = [ps_pool.tile([128, dout], F32, tag=ps_tag, name=f"{ps_tag}_{j}")
                  for j in range(CHUNK_BT)]
            for i in range(8):
                cheb = chebp.tile([128, 6, CHUNK_BT, 128], BF16, tag="cheb",
                                  name=f"cheb_{ps_tag}_{i}")
                xn = tmpp.tile([128, CHUNK_BT, 128], F32, tag="xn", name=f"xn_{i}")
                for j in range(CHUNK_BT):
                    tr = ps_tr.tile([128, 128], F32, tag="tr", name=f"tr_{i}_{j}")
                    nc.tensor.transpose(tr[:], src_tiles[j][:, i * 128:(i + 1) * 128],
                                        ident[:])
                    nc.scalar.activation(xn[:, j, :], tr[:], AF.Tanh)
                    nc.scalar.copy(cheb[:, 0, j, :], tr[:])
                _cheb_fill(nc, cheb, xn[:], tmpp, upool)
                for j in range(CHUNK_BT):
                    for d in range(6):
                        st = cheb[:, d, j, :]
                        for h in range(nbank):
                            nc.tensor.matmul(
                                ps[j][:, h * 512:(h + 1) * 512], st,
                                w_sb[:, d, i, h * 512:(h + 1) * 512],
                                start=(i == 0 and d == 0),
                                stop=(i == 7 and d == 5))
            return ps

        for c in range(NCHUNK):
            # ---- load + first tanh (in place) ----
            xt = []
            for j in range(CHUNK_BT):
                g = c * CHUNK_BT + j
                x_t = xpool.tile([128, D0], F32, tag="x", name=f"x_{g}")
                nc.sync.dma_start(out=x_t[:], in_=x_d[g * 128:(g + 1) * 128, :])
                nc.scalar.activation(x_t[:], x_t[:], AF.Tanh)
                xt.append(x_t)

            # ---- layer 1 ----
            y1ps = cheb_layer(xt, w1_sb, ps_y1, D1, "y1")

            # ---- bias + layernorm + silu ----
            hs = []
            for j in range(CHUNK_BT):
                y1 = ypool.tile([128, D1], F32, tag="y1sb", name=f"y1_{c}_{j}")
                nc.vector.tensor_add(y1[:], y1ps[j][:], b1_t[:])
                stats = statp.tile([128, 2, 6], F32, tag="stats", name="stats")
                nc.vector.bn_stats(stats[:, 0, :], y1[:, 0:512])
                nc.vector.bn_stats(stats[:, 1, :], y1[:, 512:1024])
                mv = statp.tile([128, 2], F32, tag="mv", name="mv")
                nc.vector.bn_aggr(mv[:], stats[:])
                sd = statp.tile([128, 1], F32, tag="sd", name="sd")
                nc.scalar.activation(sd[:], mv[:, 1:2], AF.Sqrt, bias=eps_t[:])
                rstd = statp.tile([128, 1], F32, tag="rstd", name="rstd")
                nc.vector.reciprocal(rstd[:], sd[:])
                nc.vector.tensor_scalar(y1[:], y1[:], mv[:, 0:1], rstd[:],
                                        op0=ALU.subtract, op1=ALU.mult)
                if g_t is not None:
                    nc.vector.tensor_mul(y1[:], y1[:], g_t[:])
                    nc.vector.tensor_add(y1[:], y1[:], be_t[:])
                nc.scalar.activation(y1[:], y1[:], AF.Silu)
                hs.append(y1)

            # ---- layer 2 ----
            y2ps = cheb_layer(hs, w2_sb, ps_y2, D2, "y2")

            for j in range(CHUNK_BT):
                g = c * CHUNK_BT + j
                o_t = opool.tile([128, D2], F32, tag="o", name=f"o_{g}")
                nc.vector.tensor_add(o_t[:], y2ps[j][:], b2_t[:])
                nc.sync.dma_start(out=out_d[g * 128:(g + 1) * 128, :], in_=o_t[:])


_PROGRAMS = {}


def _get_program(trivial_affine: bool):
    key = trivial_affine
    if key in _PROGRAMS:
        return _PROGRAMS[key]
    nc = bacc.Bacc("TRN2", target_bir_lowering=False, debug=False,
                   num_devices=N_CORES)
    x_d = nc.dram_tensor("x_in", [BC, D0], F32, kind="ExternalInput").ap()
    w1_d = nc.dram_tensor("w1", [6, D0, D1], BF16, kind="ExternalInput").ap()
    w2_d = nc.dram_tensor("w2", [6, D1, D2], BF16, kind="ExternalInput").ap()
    b1_d = nc.dram_tensor("b1e", [D1], F32, kind="ExternalInput").ap()
    b2_d = nc.dram_tensor("b2e", [D2], F32, kind="ExternalInput").ap()
    if trivial_affine:
        g_d = be_d = None
    else:
        g_d = nc.dram_tensor("gam", [D1], F32, kind="ExternalInput").ap()
        be_d = nc.dram_tensor("bet", [D1], F32, kind="ExternalInput").ap()
    out_d = nc.dram_tensor("out", [BC, D2], F32, kind="ExternalOutput").ap()

    with tile.TileContext(nc) as tc:
        _kernel_body(tc, out_d, x_d, w1_d, w2_d, b1_d, b2_d, g_d, be_d)
    nc.compile()
    _PROGRAMS[key] = nc
    return nc


def _prep_inputs(x, coeff1, base_w1, bias1, ln_gamma, ln_beta, coeff2,
                 base_w2, bias2):
    x = np.ascontiguousarray(np.asarray(x, np.float32))
    coeff1 = np.asarray(coeff1, np.float32)
    coeff2 = np.asarray(coeff2, np.float32)

    w1 = np.empty((6, D0, D1), ml_dtypes.bfloat16)
    w1[0] = np.asarray(base_w1, np.float32).T
    for d in range(1, 6):
        w1[d] = coeff1[:, :, d].T
    w2 = np.empty((6, D1, D2), ml_dtypes.bfloat16)
    w2[0] = np.asarray(base_w2, np.float32).T
    for d in range(1, 6):
        w2[d] = coeff2[:, :, d].T
    b1e = (np.asarray(bias1, np.float32)
           + coeff1[:, :, 0].sum(axis=1)).astype(np.float32)
    b2e = (np.asarray(bias2, np.float32)
           + coeff2[:, :, 0].sum(axis=1)).astype(np.float32)

    g = np.asarray(ln_gamma, np.float32)
    be = np.asarray(ln_beta, np.float32)
    trivial = bool(np.all(g == 1.0) and np.all(be == 0.0))

    shared = {"w1": w1, "w2": w2, "b1e": b1e, "b2e": b2e}
    if not trivial:
        shared["gam"] = g
        shared["bet"] = be
    in_maps = []
    for cid in range(N_CORES):
        m = dict(shared)
        m["x_in"] = np.ascontiguousarray(x[cid * BC:(cid + 1) * BC])
        in_maps.append(m)
    return trivial, in_maps


def kernel_run(trace=False, **inputs):
    trivial, in_maps = _prep_inputs(**inputs)
    nc = _get_program(trivial)
    res = run_bass_kernel_spmd(nc, in_maps, core_ids=list(range(N_CORES)),
                               trace=trace)
    out = np.concatenate([r["out"] for r in res.results], axis=0)
    return out, res


def kernel(**inputs):
    out, _ = kernel_run(trace=False, **inputs)
    return out


# revision 5
# speedup vs baseline: 1.0592x; 1.0592x over previous
"""Trainium2 Bass kernel for a 2-layer Chebyshev KAN.

Computation (degree-5 Chebyshev KAN, matching the reference):
    t1  = tanh(x)
    y1  = sum_d T_d(tanh(t1)) @ C1_d.T + t1 @ Wb1.T + b1
    h   = SiLU(LayerNorm(y1))
    out = sum_d T_d(tanh(h)) @ C2_d.T + h @ Wb2.T + b2

Strategy: data-parallel over the batch dim across 8 NeuronCores (2048 rows
per core); weights replicated, pre-transposed + cast to bf16 on the host and
kept resident in SBUF. The T_0 == 1 term is folded into an effective bias on
the host, leaving 6 [in,out] matmul matrices per layer (base + d=1..5).

On-chip dataflow per 256-row chunk (2 partition tiles of 128 rows):
  x tile -> tanh (ACT) -> PE-transpose each 128x128 block to feature-major ->
  tanh again (ACT, from PSUM) -> Chebyshev recurrence in bf16 (DVE) ->
  6 stationary [128,128] bf16 tiles per i-block -> PE matmuls accumulate
  y1[128b, 1024] in PSUM -> bias add + LayerNorm (bn_stats) + SiLU in
  batch-major layout -> PE-transpose h -> layer 2 identically -> DMA out.
"""

import math

import numpy as np
import ml_dtypes

import concourse.bass as bass
import concourse.tile as tile
from concourse import bacc, mybir
from concourse.bass_utils import run_bass_kernel_spmd
from concourse.masks import make_identity

N_CORES = 8
B, D0, D1, D2 = 16384, 1024, 1024, 512
BC = B // N_CORES            # rows per core
NBT = BC // 128              # 16 partition tiles per core
CHUNK_BT = 2                 # partition tiles processed per chunk
NCHUNK = NBT // CHUNK_BT
FD = CHUNK_BT * 128          # free-dim width of feature-major tiles
LN_EPS = 1e-5

F32 = mybir.dt.float32
BF16 = mybir.dt.bfloat16
AF = mybir.ActivationFunctionType
ALU = mybir.AluOpType


def _bcast_row(nc, pool, vec_ap, n, name):
    """Load a [n] DRAM vector broadcast across all 128 partitions."""
    t = pool.tile([128, n], F32, name=name)
    src = bass.AP(tensor=vec_ap.tensor, offset=vec_ap.offset,
                  ap=[[0, 128], list(vec_ap.ap[0])])
    nc.gpsimd.dma_start(out=t[:], in_=src)
    return t


def _cheb_fill(nc, cheb, upool):
    """Fill cheb[:, 2..5] (bf16) with T_2..T_5; cheb[:, 1] (T1 = tanh) is
    already populated.  All DVE ops are bf16 to hit the 2x/4x perf modes:
      T2 = 2*T1^2 - 1          (ACT Square + DVE tensor_scalar)
      T3 = T1 * (2*T2 - 1)
      T4 = 2*T2^2 - 1
      T5 = 2*(T2*T3) - T1
    """
    T1, T2, T3, T4, T5 = (cheb[:, k] for k in range(1, 6))

    def tmp(name):
        return upool.tile([128, CHUNK_BT, 128], BF16, tag="u", name=name)

    sq = tmp("sq")
    nc.scalar.activation(sq[:], T1, AF.Square, scale=math.sqrt(2.0))  # 2*T1^2
    nc.vector.tensor_scalar(T2, sq[:], 1.0, None, op0=ALU.subtract)
    a = tmp("a")
    nc.vector.tensor_scalar(a[:], T2, 2.0, 1.0, op0=ALU.mult, op1=ALU.subtract)
    nc.vector.tensor_tensor(T3, T1, a[:], op=ALU.mult)
    b = tmp("b")
    nc.vector.tensor_tensor(b[:], T2, T2, op=ALU.mult)
    nc.vector.tensor_scalar(T4, b[:], 2.0, 1.0, op0=ALU.mult, op1=ALU.subtract)
    c = tmp("c")
    nc.vector.tensor_tensor(c[:], T2, T3, op=ALU.mult)
    d = tmp("d")
    nc.vector.tensor_scalar(d[:], c[:], 2.0, None, op0=ALU.mult)
    nc.vector.tensor_tensor(T5, d[:], T1, op=ALU.subtract)


def _rsqrt(nc, veps, statp, magic_t):
    """1/sqrt(veps) on DVE only (bit-trick seed + 2 Newton iterations).
    veps: [128, 1] f32 (> 0).  Avoids ACT Sqrt so the whole kernel stays on
    one activation table set."""
    I32 = mybir.dt.int32
    j = statp.tile([128, 1], I32, tag="rsj", name="rsj")
    nc.vector.tensor_scalar(j[:], veps[:].bitcast(I32), 1, None,
                            op0=ALU.arith_shift_right)
    y = statp.tile([128, 1], F32, tag="rsy", name="rsy")
    nc.vector.tensor_tensor(y[:].bitcast(I32), magic_t[:], j[:], op=ALU.subtract)
    s = statp.tile([128, 1], F32, tag="rss", name="rss")
    w = statp.tile([128, 1], F32, tag="rsw", name="rsw")
    for _ in range(2):
        nc.vector.tensor_tensor(s[:], y[:], y[:], op=ALU.mult)
        nc.vector.tensor_tensor(s[:], s[:], veps[:], op=ALU.mult)
        nc.vector.tensor_scalar(w[:], s[:], -0.5, 1.5, op0=ALU.mult, op1=ALU.add)
        nc.vector.tensor_tensor(y[:], y[:], w[:], op=ALU.mult)
    return y


def _kernel_body(tc, out_d, x_d, w1_d, w2_d, b1_d, b2_d, g_d, be_d):
    nc = tc.nc
    import contextlib
    ctx = contextlib.ExitStack()
    with ctx:
        consts = ctx.enter_context(tc.tile_pool(name="consts", bufs=1))
        wpool = ctx.enter_context(tc.tile_pool(name="wpool", bufs=1))
        xpool = ctx.enter_context(tc.tile_pool(name="xpool", bufs=5))
        upool = ctx.enter_context(tc.tile_pool(name="upool", bufs=6))
        chebp = ctx.enter_context(tc.tile_pool(name="chebp", bufs=3))
        ypool = ctx.enter_context(tc.tile_pool(name="ypool", bufs=3))
        statp = ctx.enter_context(tc.tile_pool(name="statp", bufs=6))
        opool = ctx.enter_context(tc.tile_pool(name="opool", bufs=3))
        ps_acc = ctx.enter_context(tc.tile_pool(name="ps_acc", bufs=6, space="PSUM"))
        ps_tr = ctx.enter_context(tc.tile_pool(name="ps_tr", bufs=2, space="PSUM"))

        ident = consts.tile([128, 128], F32, name="ident")
        make_identity(nc, ident[:])
        magic_t = consts.tile([128, 1], mybir.dt.int32, name="magic_t")
        nc.vector.memset(magic_t[:], 0x5F3759DF)

        b1_t = _bcast_row(nc, consts, b1_d, D1, "b1_t")
        b2_t = _bcast_row(nc, consts, b2_d, D2, "b2_t")
        g_t = _bcast_row(nc, consts, g_d, D1, "g_t") if g_d is not None else None
        be_t = _bcast_row(nc, consts, be_d, D1, "be_t") if be_d is not None else None

        # Prefetch the first chunks' x tiles ahead of the weight DMAs so the
        # pipeline can start immediately.
        pre_x = {}
        for g in range(min(2 * CHUNK_BT, NBT)):
            x_t = xpool.tile([128, D0], F32, tag="x", name=f"x_{g}")
            nc.sync.dma_start(out=x_t[:], in_=x_d[g * 128:(g + 1) * 128, :])
            pre_x[g] = x_t

        # Resident weights, contraction dim (input features) on partitions.
        w1_sb = wpool.tile([128, 6, 8, D1], BF16, name="w1_sb")
        w2_sb = wpool.tile([128, 6, 8, D2], BF16, name="w2_sb")
        for i in range(8):
            for d in range(6):
                nc.sync.dma_start(out=w1_sb[:, d, i, :],
                                  in_=w1_d[d, i * 128:(i + 1) * 128, :])
        for i in range(8):
            for d in range(6):
                nc.sync.dma_start(out=w2_sb[:, d, i, :],
                                  in_=w2_d[d, i * 128:(i + 1) * 128, :])

        def cheb_layer(src_tiles, w_sb, dout, tag):
            """src_tiles: CHUNK_BT tiles [128, 1024] f32 (layer input, batch-
            major).  Returns ps[j][h]: per-batch-tile PSUM accumulators, one
            [128, 512] bank per output half."""
            nbank = dout // 512
            ps = [[ps_acc.tile([128, 512], F32, tag="acc", name=f"{tag}_{j}_{h}")
                   for h in range(nbank)] for j in range(CHUNK_BT)]
            for i in range(8):
                cheb = chebp.tile([128, 6, CHUNK_BT, 128], BF16, tag="cheb",
                                  name=f"cheb_{tag}_{i}")
                for j in range(CHUNK_BT):
                    tr = ps_tr.tile([128, 128], F32, tag="tr", name=f"tr_{i}_{j}")
                    nc.tensor.transpose(tr[:], src_tiles[j][:, i * 128:(i + 1) * 128],
                                        ident[:])
                    nc.scalar.activation(cheb[:, 1, j, :], tr[:], AF.Tanh)
                    nc.scalar.copy(cheb[:, 0, j, :], tr[:])
                _cheb_fill(nc, cheb, upool)
                for j in range(CHUNK_BT):
                    for d in range(6):
                        st = cheb[:, d, j, :]
                        for h in range(nbank):
                            nc.tensor.matmul(
                                ps[j][h][:], st,
                                w_sb[:, d, i, h * 512:(h + 1) * 512],
                                start=(i == 0 and d == 0),
                                stop=(i == 7 and d == 5))
            return ps

        for c in range(NCHUNK):
            # ---- load + first tanh (in place) ----
            xt = []
            for j in range(CHUNK_BT):
                g = c * CHUNK_BT + j
                x_t = pre_x.pop(g, None)
                if x_t is None:
                    x_t = xpool.tile([128, D0], F32, tag="x", name=f"x_{g}")
                    nc.sync.dma_start(out=x_t[:], in_=x_d[g * 128:(g + 1) * 128, :])
                nc.scalar.activation(x_t[:], x_t[:], AF.Tanh)
                xt.append(x_t)

            # ---- layer 1 ----
            y1ps = cheb_layer(xt, w1_sb, D1, f"y1_{c}")

            # ---- bias + layernorm + silu ----
            hs = []
            for j in range(CHUNK_BT):
                y1 = ypool.tile([128, D1], F32, tag="y1sb", name=f"y1_{c}_{j}")
                for h in range(2):
                    sl = slice(h * 512, (h + 1) * 512)
                    nc.vector.tensor_add(y1[:, sl], y1ps[j][h][:], b1_t[:, sl])
                stats = statp.tile([128, 2, 6], F32, tag="stats", name="stats")
                nc.vector.bn_stats(stats[:, 0, :], y1[:, 0:512])
                nc.vector.bn_stats(stats[:, 1, :], y1[:, 512:1024])
                mv = statp.tile([128, 2], F32, tag="mv", name="mv")
                nc.vector.bn_aggr(mv[:], stats[:])
                veps = statp.tile([128, 1], F32, tag="veps", name="veps")
                nc.vector.tensor_scalar(veps[:], mv[:, 1:2], LN_EPS, None,
                                        op0=ALU.add)
                rstd = _rsqrt(nc, veps, statp, magic_t)
                nc.vector.tensor_scalar(y1[:], y1[:], mv[:, 0:1], rstd[:],
                                        op0=ALU.subtract, op1=ALU.mult)
                if g_t is not None:
                    nc.vector.tensor_mul(y1[:], y1[:], g_t[:])
                    nc.vector.tensor_add(y1[:], y1[:], be_t[:])
                nc.scalar.activation(y1[:], y1[:], AF.Silu)
                hs.append(y1)

            # ---- layer 2 ----
            y2ps = cheb_layer(hs, w2_sb, D2, f"y2_{c}")

            for j in range(CHUNK_BT):
                g = c * CHUNK_BT + j
                o_t = opool.tile([128, D2], F32, tag="o", name=f"o_{g}")
                nc.vector.tensor_add(o_t[:], y2ps[j][0][:], b2_t[:])
                nc.sync.dma_start(out=out_d[g * 128:(g + 1) * 128, :], in_=o_t[:])


_PROGRAMS = {}


def _get_program(trivial_affine: bool):
    key = trivial_affine
    if key in _PROGRAMS:
        return _PROGRAMS[key]
    nc = bacc.Bacc("TRN2", target_bir_lowering=False, debug=False,
                   num_devices=N_CORES)
    x_d = nc.dram_tensor("x_in", [BC, D0], F32, kind="ExternalInput").ap()
    w1_d = nc.dram_tensor("w1", [6, D0, D1], BF16, kind="ExternalInput").ap()
    w2_d = nc.dram_tensor("w2", [6, D1, D2], BF16, kind="ExternalInput").ap()
    b1_d = nc.dram_tensor("b1e", [D1], F32, kind="ExternalInput").ap()
    b2_d = nc.dram_tensor("b2e", [D2], F32, kind="ExternalInput").ap()
    if trivial_affine:
        g_d = be_d = None
    else:
        g_d = nc.dram_tensor("gam", [D1], F32, kind="ExternalInput").ap()
        be_d = nc.dram_tensor("bet", [D1], F32, kind="ExternalInput").ap()
    out_d = nc.dram_tensor("out", [BC, D2], F32, kind="ExternalOutput").ap()

    with tile.TileContext(nc) as tc:
        _kernel_body(tc, out_d, x_d, w1_d, w2_d, b1_d, b2_d, g_d, be_d)
    nc.compile()
    _PROGRAMS[key] = nc
    return nc


def _prep_inputs(x, coeff1, base_w1, bias1, ln_gamma, ln_beta, coeff2,
                 base_w2, bias2):
    x = np.ascontiguousarray(np.asarray(x, np.float32))
    coeff1 = np.asarray(coeff1, np.float32)
    coeff2 = np.asarray(coeff2, np.float32)

    w1 = np.empty((6, D0, D1), ml_dtypes.bfloat16)
    w1[0] = np.asarray(base_w1, np.float32).T
    for d in range(1, 6):
        w1[d] = coeff1[:, :, d].T
    w2 = np.empty((6, D1, D2), ml_dtypes.bfloat16)
    w2[0] = np.asarray(base_w2, np.float32).T
    for d in range(1, 6):
        w2[d] = coeff2[:, :, d].T
    b1e = (np.asarray(bias1, np.float32)
           + coeff1[:, :, 0].sum(axis=1)).astype(np.float32)
    b2e = (np.asarray(bias2, np.float32)
           + coeff2[:, :, 0].sum(axis=1)).astype(np.float32)

    g = np.asarray(ln_gamma, np.float32)
    be = np.asarray(ln_beta, np.float32)
    trivial = bool(np.all(g == 1.0) and np.all(be == 0.0))

    shared = {"w1": w1, "w2": w2, "b1e": b1e, "b2e": b2e}
    if not trivial:
        shared["gam"] = g
        shared["bet"] = be
    in_maps = []
    for cid in range(N_CORES):
        m = dict(shared)
        m["x_in"] = np.ascontiguousarray(x[cid * BC:(cid + 1) * BC])
        in_maps.append(m)
    return trivial, in_maps


def kernel_run(trace=False, **inputs):
    trivial, in_maps = _prep_inputs(**inputs)
    nc = _get_program(trivial)
    res = run_bass_kernel_spmd(nc, in_maps, core_ids=list(range(N_CORES)),
                               trace=trace)
    out = np.concatenate([r["out"] for r in res.results], axis=0)
    return out, res


def kernel(**inputs):
    out, _ = kernel_run(trace=False, **inputs)
    return out


# revision 6
# speedup vs baseline: 1.3187x; 1.2450x over previous
"""Trainium2 Bass kernel for a 2-layer Chebyshev KAN.

Computation (degree-5 Chebyshev KAN, matching the reference):
    t1  = tanh(x)
    y1  = sum_d T_d(tanh(t1)) @ C1_d.T + t1 @ Wb1.T + b1
    h   = SiLU(LayerNorm(y1))
    out = sum_d T_d(tanh(h)) @ C2_d.T + h @ Wb2.T + b2

Strategy: data-parallel over the batch dim across 8 NeuronCores (2048 rows
per core); weights replicated, pre-transposed + cast to bf16 on the host and
kept resident in SBUF. The T_0 == 1 term is folded into an effective bias on
the host, leaving 6 [in,out] matmul matrices per layer (base + d=1..5).

On-chip dataflow per 256-row chunk (2 partition tiles of 128 rows):
  x tile -> tanh (ACT) -> PE-transpose each 128x128 block to feature-major ->
  tanh again (ACT, from PSUM) -> Chebyshev recurrence in bf16 (DVE) ->
  6 stationary [128,128] bf16 tiles per i-block -> PE matmuls accumulate
  y1[128b, 1024] in PSUM -> bias add + LayerNorm (bn_stats) + SiLU in
  batch-major layout -> PE-transpose h -> layer 2 identically -> DMA out.
"""

import math

import numpy as np
import ml_dtypes

import concourse.bass as bass
import concourse.tile as tile
from concourse import bacc, mybir
from concourse.bass_utils import run_bass_kernel_spmd
from concourse.masks import make_identity

N_CORES = 8
B, D0, D1, D2 = 16384, 1024, 1024, 512
BC = B // N_CORES            # rows per core
NBT = BC // 128              # 16 partition tiles per core
CHUNK_BT = 2                 # partition tiles processed per chunk
NCHUNK = NBT // CHUNK_BT
FD = CHUNK_BT * 128          # free-dim width of feature-major tiles
LN_EPS = 1e-5

F32 = mybir.dt.float32
BF16 = mybir.dt.bfloat16
AF = mybir.ActivationFunctionType
ALU = mybir.AluOpType


def _bcast_row(nc, pool, vec_ap, n, name):
    """Load a [n] DRAM vector broadcast across all 128 partitions."""
    t = pool.tile([128, n], F32, name=name)
    src = bass.AP(tensor=vec_ap.tensor, offset=vec_ap.offset,
                  ap=[[0, 128], list(vec_ap.ap[0])])
    nc.gpsimd.dma_start(out=t[:], in_=src)
    return t


def _cheb_fill(nc, cheb, upool):
    """Fill cheb[:, 2..5] (bf16) with T_2..T_5; cheb[:, 1] (T1 = tanh) is
    already populated.  All DVE ops are bf16 to hit the 2x/4x perf modes:
      T2 = 2*T1^2 - 1          (ACT Square + DVE tensor_scalar)
      T3 = T1 * (2*T2 - 1)
      T4 = 2*T2^2 - 1
      T5 = 2*(T2*T3) - T1
    """
    T1, T2, T3, T4, T5 = (cheb[:, k] for k in range(1, 6))

    def tmp(name):
        return upool.tile([128, CHUNK_BT, 128], BF16, tag="u", name=name)

    sq = tmp("sq")
    nc.scalar.activation(sq[:], T1, AF.Square, scale=math.sqrt(2.0))  # 2*T1^2
    nc.vector.tensor_scalar(T2, sq[:], 1.0, None, op0=ALU.subtract)
    a = tmp("a")
    nc.vector.tensor_scalar(a[:], T2, 2.0, 1.0, op0=ALU.mult, op1=ALU.subtract)
    nc.vector.tensor_tensor(T3, T1, a[:], op=ALU.mult)
    b = tmp("b")
    nc.vector.tensor_tensor(b[:], T2, T2, op=ALU.mult)
    nc.vector.tensor_scalar(T4, b[:], 2.0, 1.0, op0=ALU.mult, op1=ALU.subtract)
    c = tmp("c")
    nc.vector.tensor_tensor(c[:], T2, T3, op=ALU.mult)
    d = tmp("d")
    nc.vector.tensor_scalar(d[:], c[:], 2.0, None, op0=ALU.mult)
    nc.vector.tensor_tensor(T5, d[:], T1, op=ALU.subtract)


def _rsqrt(nc, veps, statp, magic_t):
    """1/sqrt(veps) on DVE only (bit-trick seed + 2 Newton iterations).
    veps: [128, 1] f32 (> 0).  Avoids ACT Sqrt so the whole kernel stays on
    one activation table set."""
    I32 = mybir.dt.int32
    j = statp.tile([128, 1], I32, tag="rsj", name="rsj")
    nc.vector.tensor_scalar(j[:], veps[:].bitcast(I32), 1, None,
                            op0=ALU.arith_shift_right)
    y = statp.tile([128, 1], F32, tag="rsy", name="rsy")
    nc.vector.tensor_tensor(y[:].bitcast(I32), magic_t[:], j[:], op=ALU.subtract)
    s = statp.tile([128, 1], F32, tag="rss", name="rss")
    w = statp.tile([128, 1], F32, tag="rsw", name="rsw")
    for _ in range(2):
        nc.vector.tensor_tensor(s[:], y[:], y[:], op=ALU.mult)
        nc.vector.tensor_tensor(s[:], s[:], veps[:], op=ALU.mult)
        nc.vector.tensor_scalar(w[:], s[:], -0.5, 1.5, op0=ALU.mult, op1=ALU.add)
        nc.vector.tensor_tensor(y[:], y[:], w[:], op=ALU.mult)
    return y


def _kernel_body(tc, out_d, x_d, w1_d, w2_d, b1_d, b2_d, g_d, be_d):
    nc = tc.nc
    import contextlib
    ctx = contextlib.ExitStack()
    with ctx:
        consts = ctx.enter_context(tc.tile_pool(name="consts", bufs=1))
        wpool = ctx.enter_context(tc.tile_pool(name="wpool", bufs=1))
        xpool = ctx.enter_context(tc.tile_pool(name="xpool", bufs=5))
        upool = ctx.enter_context(tc.tile_pool(name="upool", bufs=6))
        chebp = ctx.enter_context(tc.tile_pool(name="chebp", bufs=3))
        ypool = ctx.enter_context(tc.tile_pool(name="ypool", bufs=3))
        statp = ctx.enter_context(tc.tile_pool(name="statp", bufs=6))
        opool = ctx.enter_context(tc.tile_pool(name="opool", bufs=3))
        ps_acc = ctx.enter_context(tc.tile_pool(name="ps_acc", bufs=6, space="PSUM"))
        ps_tr = ctx.enter_context(tc.tile_pool(name="ps_tr", bufs=2, space="PSUM"))

        ident = consts.tile([128, 128], F32, name="ident")
        make_identity(nc, ident[:])
        magic_t = consts.tile([128, 1], mybir.dt.int32, name="magic_t")
        nc.vector.memset(magic_t[:], 0x5F3759DF)

        b1_t = _bcast_row(nc, consts, b1_d, D1, "b1_t")
        b2_t = _bcast_row(nc, consts, b2_d, D2, "b2_t")
        g_t = _bcast_row(nc, consts, g_d, D1, "g_t") if g_d is not None else None
        be_t = _bcast_row(nc, consts, be_d, D1, "be_t") if be_d is not None else None

        # Prefetch the first chunks' x tiles ahead of the weight DMAs so the
        # pipeline can start immediately.
        pre_x = {}
        for g in range(min(2 * CHUNK_BT, NBT)):
            x_t = xpool.tile([128, D0], F32, tag="x", name=f"x_{g}")
            nc.sync.dma_start(out=x_t[:], in_=x_d[g * 128:(g + 1) * 128, :])
            pre_x[g] = x_t

        # Resident weights, contraction dim (input features) on partitions.
        w1_sb = wpool.tile([128, 6, 8, D1], BF16, name="w1_sb")
        w2_sb = wpool.tile([128, 6, 8, D2], BF16, name="w2_sb")
        for i in range(8):
            for d in range(6):
                nc.sync.dma_start(out=w1_sb[:, d, i, :],
                                  in_=w1_d[d, i * 128:(i + 1) * 128, :])
        for i in range(8):
            for d in range(6):
                nc.sync.dma_start(out=w2_sb[:, d, i, :],
                                  in_=w2_d[d, i * 128:(i + 1) * 128, :])

        def cheb_layer(src_tiles, w_sb, dout, tag):
            """src_tiles: CHUNK_BT tiles [128, 1024] f32 (layer input, batch-
            major).  Returns ps[j][h]: per-batch-tile PSUM accumulators, one
            [128, 512] bank per output half.  cheb production for i-block i+1
            is emitted ahead of the matmul sweep for block i so the PE never
            waits on the ACT/DVE production chain."""
            nbank = dout // 512
            ps = [[ps_acc.tile([128, 512], F32, tag="acc", name=f"{tag}_{j}_{h}")
                   for h in range(nbank)] for j in range(CHUNK_BT)]
            chebs = [None] * 8

            def fill(i):
                cheb = chebp.tile([128, 6, CHUNK_BT, 128], BF16, tag="cheb",
                                  name=f"cheb_{tag}_{i}")
                for j in range(CHUNK_BT):
                    tr = ps_tr.tile([128, 128], F32, tag="tr", name=f"tr_{i}_{j}")
                    nc.tensor.transpose(tr[:], src_tiles[j][:, i * 128:(i + 1) * 128],
                                        ident[:])
                    nc.scalar.activation(cheb[:, 1, j, :], tr[:], AF.Tanh)
                    nc.scalar.copy(cheb[:, 0, j, :], tr[:])
                _cheb_fill(nc, cheb, upool)
                chebs[i] = cheb

            fill(0)
            for i in range(8):
                if i + 1 < 8:
                    fill(i + 1)
                for j in range(CHUNK_BT):
                    for d in range(6):
                        st = chebs[i][:, d, j, :]
                        for h in range(nbank):
                            nc.tensor.matmul(
                                ps[j][h][:], st,
                                w_sb[:, d, i, h * 512:(h + 1) * 512],
                                start=(i == 0 and d == 0),
                                stop=(i == 7 and d == 5))
            return ps

        def finish_chunk(c, y1ps):
            """LayerNorm + SiLU + layer 2 + output eviction for chunk c."""
            hs = []
            for j in range(CHUNK_BT):
                y1 = ypool.tile([128, D1], F32, tag="y1sb", name=f"y1_{c}_{j}")
                for h in range(2):
                    sl = slice(h * 512, (h + 1) * 512)
                    nc.vector.tensor_add(y1[:, sl], y1ps[j][h][:], b1_t[:, sl])
                stats = statp.tile([128, 2, 6], F32, tag="stats", name="stats")
                nc.vector.bn_stats(stats[:, 0, :], y1[:, 0:512])
                nc.vector.bn_stats(stats[:, 1, :], y1[:, 512:1024])
                mv = statp.tile([128, 2], F32, tag="mv", name="mv")
                nc.vector.bn_aggr(mv[:], stats[:])
                veps = statp.tile([128, 1], F32, tag="veps", name="veps")
                nc.vector.tensor_scalar(veps[:], mv[:, 1:2], LN_EPS, None,
                                        op0=ALU.add)
                rstd = _rsqrt(nc, veps, statp, magic_t)
                nc.vector.tensor_scalar(y1[:], y1[:], mv[:, 0:1], rstd[:],
                                        op0=ALU.subtract, op1=ALU.mult)
                if g_t is not None:
                    nc.vector.tensor_mul(y1[:], y1[:], g_t[:])
                    nc.vector.tensor_add(y1[:], y1[:], be_t[:])
                nc.scalar.activation(y1[:], y1[:], AF.Silu)
                hs.append(y1)

            y2ps = cheb_layer(hs, w2_sb, D2, f"y2_{c}")

            for j in range(CHUNK_BT):
                g = c * CHUNK_BT + j
                o_t = opool.tile([128, D2], F32, tag="o", name=f"o_{g}")
                nc.vector.tensor_add(o_t[:], y2ps[j][0][:], b2_t[:])
                nc.sync.dma_start(out=out_d[g * 128:(g + 1) * 128, :], in_=o_t[:])

        # Software-pipelined: layer 1 of chunk c runs on the PE while the
        # serial LayerNorm chain + layer 2 of chunk c-1 complete.
        pending = None
        for c in range(NCHUNK):
            xt = []
            for j in range(CHUNK_BT):
                g = c * CHUNK_BT + j
                x_t = pre_x.pop(g, None)
                if x_t is None:
                    x_t = xpool.tile([128, D0], F32, tag="x", name=f"x_{g}")
                    nc.sync.dma_start(out=x_t[:], in_=x_d[g * 128:(g + 1) * 128, :])
                nc.scalar.activation(x_t[:], x_t[:], AF.Tanh)
                xt.append(x_t)

            y1ps = cheb_layer(xt, w1_sb, D1, f"y1_{c}")
            if pending is not None:
                finish_chunk(*pending)
            pending = (c, y1ps)
        finish_chunk(*pending)


_PROGRAMS = {}


def _get_program(trivial_affine: bool):
    key = trivial_affine
    if key in _PROGRAMS:
        return _PROGRAMS[key]
    nc = bacc.Bacc("TRN2", target_bir_lowering=False, debug=False,
                   num_devices=N_CORES)
    x_d = nc.dram_tensor("x_in", [BC, D0], F32, kind="ExternalInput").ap()
    w1_d = nc.dram_tensor("w1", [6, D0, D1], BF16, kind="ExternalInput").ap()
    w2_d = nc.dram_tensor("w2", [6, D1, D2], BF16, kind="ExternalInput").ap()
    b1_d = nc.dram_tensor("b1e", [D1], F32, kind="ExternalInput").ap()
    b2_d = nc.dram_tensor("b2e", [D2], F32, kind="ExternalInput").ap()
    if trivial_affine:
        g_d = be_d = None
    else:
        g_d = nc.dram_tensor("gam", [D1], F32, kind="ExternalInput").ap()
        be_d = nc.dram_tensor("bet", [D1], F32, kind="ExternalInput").ap()
    out_d = nc.dram_tensor("out", [BC, D2], F32, kind="ExternalOutput").ap()

    with tile.TileContext(nc) as tc:
        _kernel_body(tc, out_d, x_d, w1_d, w2_d, b1_d, b2_d, g_d, be_d)
    nc.compile()
    _PROGRAMS[key] = nc
    return nc


def _prep_inputs(x, coeff1, base_w1, bias1, ln_gamma, ln_beta, coeff2,
                 base_w2, bias2):
    x = np.ascontiguousarray(np.asarray(x, np.float32))
    coeff1 = np.asarray(coeff1, np.float32)
    coeff2 = np.asarray(coeff2, np.float32)

    w1 = np.empty((6, D0, D1), ml_dtypes.bfloat16)
    w1[0] = np.asarray(base_w1, np.float32).T
    for d in range(1, 6):
        w1[d] = coeff1[:, :, d].T
    w2 = np.empty((6, D1, D2), ml_dtypes.bfloat16)
    w2[0] = np.asarray(base_w2, np.float32).T
    for d in range(1, 6):
        w2[d] = coeff2[:, :, d].T
    b1e = (np.asarray(bias1, np.float32)
           + coeff1[:, :, 0].sum(axis=1)).astype(np.float32)
    b2e = (np.asarray(bias2, np.float32)
           + coeff2[:, :, 0].sum(axis=1)).astype(np.float32)

    g = np.asarray(ln_gamma, np.float32)
    be = np.asarray(ln_beta, np.float32)
    trivial = bool(np.all(g == 1.0) and np.all(be == 0.0))

    shared = {"w1": w1, "w2": w2, "b1e": b1e, "b2e": b2e}
    if not trivial:
        shared["gam"] = g
        shared["bet"] = be
    in_maps = []
    for cid in range(N_CORES):
        m = dict(shared)
        m["x_in"] = np.ascontiguousarray(x[cid * BC:(cid + 1) * BC])
        in_maps.append(m)
    return trivial, in_maps


def kernel_run(trace=False, **inputs):
    trivial, in_maps = _prep_inputs(**inputs)
    nc = _get_program(trivial)
    res = run_bass_kernel_spmd(nc, in_maps, core_ids=list(range(N_CORES)),
                               trace=trace)
    out = np.concatenate([r["out"] for r in res.results], axis=0)
    return out, res


def kernel(**inputs):
    out, _ = kernel_run(trace=False, **inputs)
    return out


# revision 10
# speedup vs baseline: 1.3372x; 1.0140x over previous
"""Trainium2 Bass kernel for a 2-layer Chebyshev KAN.

Computation (degree-5 Chebyshev KAN, matching the reference):
    t1  = tanh(x)
    y1  = sum_d T_d(tanh(t1)) @ C1_d.T + t1 @ Wb1.T + b1
    h   = SiLU(LayerNorm(y1))
    out = sum_d T_d(tanh(h)) @ C2_d.T + h @ Wb2.T + b2

Strategy: data-parallel over the batch dim across 8 NeuronCores (2048 rows
per core); weights replicated, pre-transposed + cast to bf16 on the host and
kept resident in SBUF. The T_0 == 1 term is folded into an effective bias on
the host, leaving 6 [in,out] matmul matrices per layer (base + d=1..5).

On-chip dataflow per 256-row chunk (2 partition tiles of 128 rows):
  x tile -> tanh (ACT) -> PE-transpose each 128x128 block to feature-major ->
  tanh again (ACT, from PSUM) -> Chebyshev recurrence in bf16 (DVE) ->
  6 stationary [128,128] bf16 tiles per i-block -> PE matmuls accumulate
  y1[128b, 1024] in PSUM -> bias add + LayerNorm (bn_stats) + SiLU in
  batch-major layout -> PE-transpose h -> layer 2 identically -> DMA out.
"""

import math

import numpy as np
import ml_dtypes

import concourse.bass as bass
import concourse.tile as tile
from concourse import bacc, mybir
from concourse.bass_utils import run_bass_kernel_spmd
from concourse.masks import make_identity

N_CORES = 8
B, D0, D1, D2 = 16384, 1024, 1024, 512
BC = B // N_CORES            # rows per core
NBT = BC // 128              # 16 partition tiles per core
CHUNK_BT = 2                 # partition tiles processed per chunk
NCHUNK = NBT // CHUNK_BT
FD = CHUNK_BT * 128          # free-dim width of feature-major tiles
LN_EPS = 1e-5

F32 = mybir.dt.float32
BF16 = mybir.dt.bfloat16
AF = mybir.ActivationFunctionType
ALU = mybir.AluOpType


def _bcast_row(nc, pool, vec_ap, n, name):
    """Load a [n] DRAM vector broadcast across all 128 partitions."""
    t = pool.tile([128, n], F32, name=name)
    src = bass.AP(tensor=vec_ap.tensor, offset=vec_ap.offset,
                  ap=[[0, 128], list(vec_ap.ap[0])])
    nc.gpsimd.dma_start(out=t[:], in_=src)
    return t


def _cheb_fill(nc, cheb, upool):
    """Fill cheb[:, 2..5] (bf16) with T_2..T_5; cheb[:, 1] (T1 = tanh) is
    already populated.  All DVE ops are bf16 to hit the 2x/4x perf modes:
      T2 = 2*T1^2 - 1          (ACT Square + DVE tensor_scalar)
      T3 = T1 * (2*T2 - 1)
      T4 = 2*T2^2 - 1
      T5 = 2*(T2*T3) - T1
    """
    T1, T2, T3, T4, T5 = (cheb[:, k] for k in range(1, 6))

    def tmp(name):
        return upool.tile([128, CHUNK_BT, 128], BF16, tag="u", name=name)

    sq = tmp("sq")
    nc.scalar.activation(sq[:], T1, AF.Square, scale=math.sqrt(2.0))  # 2*T1^2
    nc.vector.tensor_scalar(T2, sq[:], 1.0, None, op0=ALU.subtract)
    a = tmp("a")
    nc.vector.tensor_scalar(a[:], T2, 2.0, 1.0, op0=ALU.mult, op1=ALU.subtract)
    nc.vector.tensor_tensor(T3, T1, a[:], op=ALU.mult)
    b = tmp("b")
    nc.vector.tensor_tensor(b[:], T2, T2, op=ALU.mult)
    nc.vector.tensor_scalar(T4, b[:], 2.0, 1.0, op0=ALU.mult, op1=ALU.subtract)
    c = tmp("c")
    nc.vector.tensor_tensor(c[:], T2, T3, op=ALU.mult)
    d = tmp("d")
    nc.vector.tensor_scalar(d[:], c[:], 2.0, None, op0=ALU.mult)
    nc.vector.tensor_tensor(T5, d[:], T1, op=ALU.subtract)


def _rsqrt(nc, veps, statp, magic_t):
    """1/sqrt(veps) on DVE only (bit-trick seed + 2 Newton iterations).
    veps: [128, 1] f32 (> 0).  Avoids ACT Sqrt so the whole kernel stays on
    one activation table set."""
    I32 = mybir.dt.int32
    j = statp.tile([128, 1], I32, tag="rsj", name="rsj")
    nc.vector.tensor_scalar(j[:], veps[:].bitcast(I32), 1, None,
                            op0=ALU.arith_shift_right)
    y = statp.tile([128, 1], F32, tag="rsy", name="rsy")
    nc.vector.tensor_tensor(y[:].bitcast(I32), magic_t[:], j[:], op=ALU.subtract)
    s = statp.tile([128, 1], F32, tag="rss", name="rss")
    w = statp.tile([128, 1], F32, tag="rsw", name="rsw")
    for _ in range(2):
        nc.vector.tensor_tensor(s[:], y[:], y[:], op=ALU.mult)
        nc.vector.tensor_tensor(s[:], s[:], veps[:], op=ALU.mult)
        nc.vector.tensor_scalar(w[:], s[:], -0.5, 1.5, op0=ALU.mult, op1=ALU.add)
        nc.vector.tensor_tensor(y[:], y[:], w[:], op=ALU.mult)
    return y


def _kernel_body(tc, out_d, x_d, w1_d, w2_d, b1_d, b2_d, g_d, be_d):
    nc = tc.nc
    import contextlib
    ctx = contextlib.ExitStack()
    with ctx:
        consts = ctx.enter_context(tc.tile_pool(name="consts", bufs=1))
        wpool = ctx.enter_context(tc.tile_pool(name="wpool", bufs=1))
        xpool = ctx.enter_context(tc.tile_pool(name="xpool", bufs=5))
        upool = ctx.enter_context(tc.tile_pool(name="upool", bufs=8))
        chebp = ctx.enter_context(tc.tile_pool(name="chebp", bufs=4))
        ypool = ctx.enter_context(tc.tile_pool(name="ypool", bufs=3))
        statp = ctx.enter_context(tc.tile_pool(name="statp", bufs=6))
        opool = ctx.enter_context(tc.tile_pool(name="opool", bufs=3))
        ps_acc = ctx.enter_context(tc.tile_pool(name="ps_acc", bufs=6, space="PSUM"))
        ps_tr = ctx.enter_context(tc.tile_pool(name="ps_tr", bufs=2, space="PSUM"))

        ident = consts.tile([128, 128], F32, name="ident")
        make_identity(nc, ident[:])
        magic_t = consts.tile([128, 1], mybir.dt.int32, name="magic_t")
        nc.vector.memset(magic_t[:], 0x5F3759DF)
        # Trigger the (single) ACT table-set load while the first DMAs are in
        # flight: Silu selects silu_and_others, which also covers Tanh/Square/
        # Copy -- the only ACT functions this kernel uses.
        warm = consts.tile([128, 1], F32, name="warm")
        nc.scalar.activation(warm[:], magic_t[:].bitcast(F32), AF.Silu)

        b1_t = _bcast_row(nc, consts, b1_d, D1, "b1_t")
        b2_t = _bcast_row(nc, consts, b2_d, D2, "b2_t")
        g_t = _bcast_row(nc, consts, g_d, D1, "g_t") if g_d is not None else None
        be_t = _bcast_row(nc, consts, be_d, D1, "be_t") if be_d is not None else None

        # Prefetch the first chunks' x tiles ahead of the weight DMAs so the
        # pipeline can start immediately.
        pre_x = {}
        for g in range(min(2 * CHUNK_BT, NBT)):
            x_t = xpool.tile([128, D0], F32, tag="x", name=f"x_{g}")
            if g < CHUNK_BT:
                # Quarter the first chunk's loads so the tanh -> transpose
                # pipeline starts as soon as the first columns land.
                for q in range(4):
                    nc.sync.dma_start(out=x_t[:, q * 256:(q + 1) * 256],
                                      in_=x_d[g * 128:(g + 1) * 128,
                                              q * 256:(q + 1) * 256])
            else:
                nc.sync.dma_start(out=x_t[:], in_=x_d[g * 128:(g + 1) * 128, :])
            pre_x[g] = x_t

        # Resident weights, contraction dim (input features) on partitions.
        w1_sb = wpool.tile([128, 6, 8, D1], BF16, name="w1_sb")
        w2_sb = wpool.tile([128, 6, 8, D2], BF16, name="w2_sb")
        for i in range(8):
            for d in range(6):
                nc.sync.dma_start(out=w1_sb[:, d, i, :],
                                  in_=w1_d[d, i * 128:(i + 1) * 128, :])
        for i in range(8):
            for d in range(6):
                nc.sync.dma_start(out=w2_sb[:, d, i, :],
                                  in_=w2_d[d, i * 128:(i + 1) * 128, :])

        def cheb_layer(src_tiles, w_sb, dout, tag):
            """src_tiles: CHUNK_BT tiles [128, 1024] f32 (layer input, batch-
            major).  Returns ps[j][h]: per-batch-tile PSUM accumulators, one
            [128, 512] bank per output half.  cheb production for i-block i+1
            is emitted ahead of the matmul sweep for block i so the PE never
            waits on the ACT/DVE production chain."""
            nbank = dout // 512
            ps = [[ps_acc.tile([128, 512], F32, tag="acc", name=f"{tag}_{j}_{h}")
                   for h in range(nbank)] for j in range(CHUNK_BT)]
            chebs = [None] * 8

            def fill(i):
                cheb = chebp.tile([128, 6, CHUNK_BT, 128], BF16, tag="cheb",
                                  name=f"cheb_{tag}_{i}")
                for j in range(CHUNK_BT):
                    tr = ps_tr.tile([128, 128], F32, tag="tr", name=f"tr_{i}_{j}")
                    nc.tensor.transpose(tr[:], src_tiles[j][:, i * 128:(i + 1) * 128],
                                        ident[:])
                    nc.scalar.activation(cheb[:, 1, j, :], tr[:], AF.Tanh)
                    nc.any.tensor_copy(cheb[:, 0, j, :], tr[:])
                _cheb_fill(nc, cheb, upool)
                chebs[i] = cheb

            fill(0)
            for i in range(8):
                if i + 1 < 8:
                    fill(i + 1)
                for j in range(CHUNK_BT):
                    for d in range(6):
                        st = chebs[i][:, d, j, :]
                        for h in range(nbank):
                            nc.tensor.matmul(
                                ps[j][h][:], st,
                                w_sb[:, d, i, h * 512:(h + 1) * 512],
                                start=(i == 0 and d == 0),
                                stop=(i == 7 and d == 5))
            return ps

        def finish_chunk(c, y1ps):
            """LayerNorm + SiLU + layer 2 + output eviction for chunk c."""
            hs = []
            for j in range(CHUNK_BT):
                y1 = ypool.tile([128, D1], F32, tag="y1sb", name=f"y1_{c}_{j}")
                for h in range(2):
                    sl = slice(h * 512, (h + 1) * 512)
                    nc.vector.tensor_add(y1[:, sl], y1ps[j][h][:], b1_t[:, sl])
                stats = statp.tile([128, 2, 6], F32, tag="stats", name="stats")
                nc.vector.bn_stats(stats[:, 0, :], y1[:, 0:512])
                nc.vector.bn_stats(stats[:, 1, :], y1[:, 512:1024])
                mv = statp.tile([128, 2], F32, tag="mv", name="mv")
                nc.vector.bn_aggr(mv[:], stats[:])
                veps = statp.tile([128, 1], F32, tag="veps", name="veps")
                nc.vector.tensor_scalar(veps[:], mv[:, 1:2], LN_EPS, None,
                                        op0=ALU.add)
                rstd = _rsqrt(nc, veps, statp, magic_t)
                nc.vector.tensor_scalar(y1[:], y1[:], mv[:, 0:1], rstd[:],
                                        op0=ALU.subtract, op1=ALU.mult)
                if g_t is not None:
                    nc.vector.tensor_mul(y1[:], y1[:], g_t[:])
                    nc.vector.tensor_add(y1[:], y1[:], be_t[:])
                nc.scalar.activation(y1[:], y1[:], AF.Silu)
                hs.append(y1)

            y2ps = cheb_layer(hs, w2_sb, D2, f"y2_{c}")

            for j in range(CHUNK_BT):
                g = c * CHUNK_BT + j
                o_t = opool.tile([128, D2], F32, tag="o", name=f"o_{g}")
                nc.vector.tensor_add(o_t[:], y2ps[j][0][:], b2_t[:])
                nc.sync.dma_start(out=out_d[g * 128:(g + 1) * 128, :], in_=o_t[:])

        # Software-pipelined: layer 1 of chunk c runs on the PE while the
        # serial LayerNorm chain + layer 2 of chunk c-1 complete.
        pending = None
        for c in range(NCHUNK):
            xt = []
            for j in range(CHUNK_BT):
                g = c * CHUNK_BT + j
                x_t = pre_x.pop(g, None)
                if x_t is None:
                    x_t = xpool.tile([128, D0], F32, tag="x", name=f"x_{g}")
                    nc.sync.dma_start(out=x_t[:], in_=x_d[g * 128:(g + 1) * 128, :])
                nc.scalar.activation(x_t[:], x_t[:], AF.Tanh)
                xt.append(x_t)

            y1ps = cheb_layer(xt, w1_sb, D1, f"y1_{c}")
            if pending is not None:
                finish_chunk(*pending)
            pending = (c, y1ps)
        finish_chunk(*pending)


_PROGRAMS = {}


def _get_program(trivial_affine: bool):
    key = trivial_affine
    if key in _PROGRAMS:
        return _PROGRAMS[key]
    nc = bacc.Bacc("TRN2", target_bir_lowering=False, debug=False,
                   num_devices=N_CORES)
    x_d = nc.dram_tensor("x_in", [BC, D0], F32, kind="ExternalInput").ap()
    w1_d = nc.dram_tensor("w1", [6, D0, D1], BF16, kind="ExternalInput").ap()
    w2_d = nc.dram_tensor("w2", [6, D1, D2], BF16, kind="ExternalInput").ap()
    b1_d = nc.dram_tensor("b1e", [D1], F32, kind="ExternalInput").ap()
    b2_d = nc.dram_tensor("b2e", [D2], F32, kind="ExternalInput").ap()
    if trivial_affine:
        g_d = be_d = None
    else:
        g_d = nc.dram_tensor("gam", [D1], F32, kind="ExternalInput").ap()
        be_d = nc.dram_tensor("bet", [D1], F32, kind="ExternalInput").ap()
    out_d = nc.dram_tensor("out", [BC, D2], F32, kind="ExternalOutput").ap()

    with tile.TileContext(nc) as tc:
        _kernel_body(tc, out_d, x_d, w1_d, w2_d, b1_d, b2_d, g_d, be_d)
    nc.compile()
    _PROGRAMS[key] = nc
    return nc


def _prep_inputs(x, coeff1, base_w1, bias1, ln_gamma, ln_beta, coeff2,
                 base_w2, bias2):
    x = np.ascontiguousarray(np.asarray(x, np.float32))
    coeff1 = np.asarray(coeff1, np.float32)
    coeff2 = np.asarray(coeff2, np.float32)

    w1 = np.empty((6, D0, D1), ml_dtypes.bfloat16)
    w1[0] = np.asarray(base_w1, np.float32).T
    for d in range(1, 6):
        w1[d] = coeff1[:, :, d].T
    w2 = np.empty((6, D1, D2), ml_dtypes.bfloat16)
    w2[0] = np.asarray(base_w2, np.float32).T
    for d in range(1, 6):
        w2[d] = coeff2[:, :, d].T
    b1e = (np.asarray(bias1, np.float32)
           + coeff1[:, :, 0].sum(axis=1)).astype(np.float32)
    b2e = (np.asarray(bias2, np.float32)
           + coeff2[:, :, 0].sum(axis=1)).astype(np.float32)

    g = np.asarray(ln_gamma, np.float32)
    be = np.asarray(ln_beta, np.float32)
    trivial = bool(np.all(g == 1.0) and np.all(be == 0.0))

    shared = {"w1": w1, "w2": w2, "b1e": b1e, "b2e": b2e}
    if not trivial:
        shared["gam"] = g
        shared["bet"] = be
    in_maps = []
    for cid in range(N_CORES):
        m = dict(shared)
        m["x_in"] = np.ascontiguousarray(x[cid * BC:(cid + 1) * BC])
        in_maps.append(m)
    return trivial, in_maps


def kernel_run(trace=False, **inputs):
    trivial, in_maps = _prep_inputs(**inputs)
    nc = _get_program(trivial)
    res = run_bass_kernel_spmd(nc, in_maps, core_ids=list(range(N_CORES)),
                               trace=trace)
    out = np.concatenate([r["out"] for r in res.results], axis=0)
    return out, res


def kernel(**inputs):
    out, _ = kernel_run(trace=False, **inputs)
    return out
